# revision 2
# baseline (speedup 1.0000x reference)
"""Trainium2 Bass kernel for nn_MultiHeadAttention (B=2,S=2048,D=1024,H=16, RoPE+ALiBi+causal).

Head-parallel sharding across 8 NeuronCores (2 heads/core, both batches).
v3:
  - Interleaved emission: attention block j runs while projection groups j+1
    are in flight; output projection of block j is emitted after group j+1 so
    the softmax-normalization chain hides under projection matmuls.
  - ALiBi+mask folded into per-head band matrices B[kl, c] = exp(slope*(kl-c)),
    SBUF-resident; head pairing (full + 12-tile-window head) per core.
  - reciprocal_approx_fast + GpSimd partition_broadcast normalization.
  - Const loads on the scalar HWDGE queue, outputs on the GpSimd SWDGE queue,
    x loads on the sync queue (three parallel DMA issue paths).
  - Partial outputs bf16, summed fp32 on host.
"""
import sys, os

for _p in ("/root/.axon_site/_ro/trn_rl_repo", "/opt/trn_rl_repo"):
    if os.path.isdir(_p) and _p not in sys.path:
        sys.path.insert(0, _p)

import numpy as np
import ml_dtypes
import concourse.bass as bass
import concourse.mybir as mybir
import concourse.tile as tile
from concourse import bacc
from concourse.bass_utils import run_bass_kernel_spmd

F32 = mybir.dt.float32
BF16 = mybir.dt.bfloat16

B, S, D, H = 2, 2048, 1024, 16
DK = D // H            # 64
NCORES = 8
HPC = H // NCORES      # 2 heads per core
FD = HPC * DK          # 128 ctx features per core
R = B * S              # 4096 token rows
RT = R // 128          # 32 r-tiles
QB = 512               # q-block size
NQB = S // QB          # 4 q-blocks per batch
NSEG = NQB             # token segments per batch (512 tokens each)
WIN1 = 12              # k-tile window for the slot-1 (fast-decay) head


def _build(causal: bool, qk_bias: bool):
    nc = bacc.Bacc()

    xT = nc.dram_tensor("xT", (D, R), BF16, kind="ExternalInput")
    wcat = nc.dram_tensor("wcat", (D, 3 * FD), BF16, kind="ExternalInput")
    woB = nc.dram_tensor("woB", (2 * DK, D), BF16, kind="ExternalInput")
    bband = nc.dram_tensor("bband", (HPC, 128, S), BF16, kind="ExternalInput")
    cosp = nc.dram_tensor("cosp", (128, RT * (DK // 2)), BF16, kind="ExternalInput")
    sinp = nc.dram_tensor("sinp", (128, RT * (DK // 2)), BF16, kind="ExternalInput")
    ident = nc.dram_tensor("ident", (128, 128), BF16, kind="ExternalInput")
    identf = nc.dram_tensor("identf", (128, 128), F32, kind="ExternalInput")
    if qk_bias:
        bropeq = nc.dram_tensor("bropeq", (128, R), BF16, kind="ExternalInput")
        bropek = nc.dram_tensor("bropek", (128, R), BF16, kind="ExternalInput")
    out = nc.dram_tensor("out", (R, D), BF16, kind="ExternalOutput")

    with tile.TileContext(nc) as tc:
        import contextlib
        ctx = contextlib.ExitStack()
        with ctx:
            consts = ctx.enter_context(tc.tile_pool(name="consts", bufs=1))
            persist = ctx.enter_context(tc.tile_pool(name="persist", bufs=1))
            p1x = ctx.enter_context(tc.tile_pool(name="p1x", bufs=2))
            p1n = ctx.enter_context(tc.tile_pool(name="p1n", bufs=2))
            p1s = ctx.enter_context(tc.tile_pool(name="p1s", bufs=2))
            p2e = ctx.enter_context(tc.tile_pool(name="p2e", bufs=3))
            p2a = ctx.enter_context(tc.tile_pool(name="p2a", bufs=3))
            p2c = ctx.enter_context(tc.tile_pool(name="p2c", bufs=2))
            p2r = ctx.enter_context(tc.tile_pool(name="p2r", bufs=2))
            p2o = ctx.enter_context(tc.tile_pool(name="p2o", bufs=3))
            psc = ctx.enter_context(tc.tile_pool(name="psc", bufs=2, space="PSUM"))
            psx = ctx.enter_context(tc.tile_pool(name="psx", bufs=1, space="PSUM"))
            psh = ctx.enter_context(tc.tile_pool(name="psh", bufs=2, space="PSUM"))

            # --- constants (issued on the scalar HWDGE queue; x loads use sync) ---
            id_sb = consts.tile([128, 128], BF16)
            nc.scalar.dma_start(out=id_sb, in_=ident[:, :])
            idf_sb = consts.tile([128, 128], F32, tag="idf", name="idf")
            nc.scalar.dma_start(out=idf_sb, in_=identf[:, :])
            wc_sb = [consts.tile([128, 3 * FD], BF16, tag=f"wc{ct}", name=f"wc{ct}") for ct in range(8)]
            for ct in range(8):
                nc.scalar.dma_start(out=wc_sb[ct], in_=wcat[ct * 128:(ct + 1) * 128, :])
            cos_sb = consts.tile([128, RT * 32], BF16)
            sin_sb = consts.tile([128, RT * 32], BF16)
            nc.scalar.dma_start(out=cos_sb, in_=cosp[:, :])
            nc.scalar.dma_start(out=sin_sb, in_=sinp[:, :])
            wo_sb = consts.tile([2 * DK, D], BF16, tag="wo", name="wo")
            nc.scalar.dma_start(out=wo_sb, in_=woB[:, :])
            bb_sb = [consts.tile([128, S], BF16, tag=f"bb{h}", name=f"bb{h}") for h in range(HPC)]
            for h in range(HPC):
                nc.scalar.dma_start(out=bb_sb[h], in_=bband[h, :, :])

            # --- persistent activation tensors ---
            QT = {}
            KT = {}
            for b in range(B):
                for j in range(NSEG):
                    QT[(b, j)] = persist.tile([128, QB], BF16, tag=f"QT{b}{j}", name=f"QT{b}{j}")
                    KT[(b, j)] = persist.tile([128, QB], BF16, tag=f"KT{b}{j}", name=f"KT{b}{j}")
            vaug_t = {}
            for rt in range(RT):
                for hh in range(HPC):
                    vaug_t[(rt, hh)] = persist.tile([128, DK + 1], BF16,
                                                    tag=f"va{rt}_{hh}", name=f"va{rt}_{hh}")
                    nc.gpsimd.memset(vaug_t[(rt, hh)][:, DK:DK + 1], 1.0)

            def vaug(rt, hh):
                return vaug_t[(rt, hh)]

            if qk_bias:
                brq = consts.tile([128, R], BF16, tag="brq")
                brk = consts.tile([128, R], BF16, tag="brk")
                nc.scalar.dma_start(out=brq, in_=bropeq[:, :])
                nc.scalar.dma_start(out=brk, in_=bropek[:, :])

            # ---------------- phase-1 group: issues x loads now, returns work units ----------------
            def make_group_units(b, j):
                rt0 = b * (S // 128) + 4 * j
                xg = [p1x.tile([128, 512], BF16, tag=f"x{ct}", name=f"xg{b}{j}_{ct}")
                      for ct in range(8)]
                for ct in range(8):
                    nc.sync.dma_start(
                        out=xg[ct],
                        in_=xT[ct * 128:(ct + 1) * 128, rt0 * 128: rt0 * 128 + 512])
                qknat = p1n.tile([128, 4 * 256], BF16, tag="qkn", name=f"qkn{b}{j}")
                qkrot = p1n.tile([128, 4 * 256], BF16, tag="qkr", name=f"qkr{b}{j}")

                def proj_unit(i):
                    def emit():
                        rt = rt0 + i
                        pp = psh.tile([128, 3 * FD], F32, tag="sh", name=f"prj{b}{j}{i}")
                        for ct in range(8):
                            nc.tensor.matmul(pp, xg[ct][:, i * 128:(i + 1) * 128], wc_sb[ct],
                                             start=(ct == 0), stop=(ct == 7))
                        nc.scalar.copy(qknat[:, i * 256: i * 256 + 256], pp[:, 0:256])
                        nc.vector.tensor_copy(vaug_t[(rt, 0)][:, 0:DK],
                                              pp[:, 2 * FD: 2 * FD + DK])
                        nc.vector.tensor_copy(vaug_t[(rt, 1)][:, 0:DK],
                                              pp[:, 2 * FD + DK: 2 * FD + 2 * DK])
                    return emit

                # rope (deinterleaved pairs: 32 even | 32 odd per head)
                def sl(t, qk, eo):
                    a = t[:, :]
                    return bass.AP(
                        tensor=a.tensor, offset=a.offset + qk * 128 + eo * 32,
                        ap=[a.ap[0], [256, 4], [64, 2], [1, 32]])
                def slc(t):
                    a = t[:, :]
                    return bass.AP(
                        tensor=a.tensor, offset=a.offset + rt0 * 32,
                        ap=[a.ap[0], [32, 4], [0, 2], [1, 32]])

                def rope_unit(qk):
                    def emit():
                        s1 = p1s.tile([128, 4 * 64], BF16, tag="s1")
                        s2 = p1s.tile([128, 4 * 64], BF16, tag="s2")
                        s3 = p1s.tile([128, 4 * 64], BF16, tag="s3")
                        s4 = p1s.tile([128, 4 * 64], BF16, tag="s4")
                        nc.vector.tensor_mul(s1, sl(qknat, qk, 0), slc(cos_sb))
                        nc.vector.tensor_mul(s2, sl(qknat, qk, 1), slc(sin_sb))
                        nc.vector.tensor_sub(sl(qkrot, qk, 0), s1, s2)
                        nc.vector.tensor_mul(s3, sl(qknat, qk, 0), slc(sin_sb))
                        nc.vector.tensor_mul(s4, sl(qknat, qk, 1), slc(cos_sb))
                        nc.vector.tensor_add(sl(qkrot, qk, 1), s3, s4)
                    return emit

                def tr_unit(i):
                    def emit():
                        for qk, dstm in ((0, QT), (1, KT)):
                            pt = psh.tile([128, 128], BF16, tag="sh", name=f"pt{b}{j}{i}{qk}")
                            nc.tensor.transpose(
                                pt, qkrot[:, i * 256 + qk * 128: i * 256 + qk * 128 + 128], id_sb)
                            dst = dstm[(b, j)][:, i * 128:(i + 1) * 128]
                            if qk == 0:
                                nc.scalar.copy(dst, pt)
                            else:
                                nc.vector.tensor_copy(dst, pt)
                        if qk_bias and i == 3:
                            for src, dstm in ((brq, QT), (brk, KT)):
                                d = dstm[(b, j)]
                                nc.vector.tensor_add(d, d, src[:, rt0 * 128: rt0 * 128 + 512])
                    return emit

                return ([proj_unit(i) for i in range(4)]
                        + [rope_unit(0), rope_unit(1)]
                        + [tr_unit(i) for i in range(4)])

            # ---------------- attention block qb: scores/softmax/ctx + norm ----------------
            def emit_qb_attn(qb, units):
                nkt = (qb + 1) * (QB // 128) if causal else S // 128
                kt_total = nkt + (nkt - max(0, nkt - WIN1))
                popped = [0]
                ki = [0]

                def pump():
                    # feed next-segment projection units evenly across the loop
                    target = min(len(units), len(units) * (ki[0] + 1) // max(1, kt_total))
                    while popped[0] < target:
                        units[popped[0]]()
                        popped[0] += 1
                    ki[0] += 1

                csc = {}
                for b in range(B):
                    csc[b] = p2c.tile([2 * DK, QB], BF16, tag=f"cb{b}", name=f"cb{qb}{b}")
                for hh in range(HPC):
                    kt_lo = 0 if hh == 0 else max(0, nkt - WIN1)
                    ctx_ps = {}
                    for b in range(B):
                        ctx_ps[b] = psx.tile([DK + 1, QB], F32, tag=f"ctx{b}", name=f"ctx{qb}_{hh}{b}")
                    pend = None
                    def issue_ctx(pkt, p_off, p_len, p_at):
                        for b in range(B):
                            nc.tensor.matmul(
                                ctx_ps[b][:, p_off:QB],
                                vaug(b * (S // 128) + pkt, hh),
                                p_at[:, b * QB: b * QB + p_len],
                                start=(pkt == kt_lo), stop=(pkt == nkt - 1))
                    for kt in range(kt_lo, nkt):
                        pump()
                        q_off = max(0, kt * 128 - qb * QB) if causal else 0
                        q_len = QB - q_off
                        scp = psc.tile([128, 2 * QB], F32, tag="sc", name=f"sc{qb}_{hh}_{kt}")
                        for b in range(B):
                            nc.tensor.matmul(
                                scp[:, b * QB: b * QB + q_len],
                                KT[(b, kt // 4)][hh * DK:(hh + 1) * DK,
                                                 (kt % 4) * 128: (kt % 4) * 128 + 128],
                                QT[(b, qb)][hh * DK:(hh + 1) * DK, q_off:QB],
                                start=True, stop=True)
                        if pend is not None:
                            issue_ctx(*pend)
                        ex = p2e.tile([128, 2 * QB], BF16, tag="ex", name=f"ex{qb}_{hh}_{kt}")
                        nc.scalar.activation(ex[:, 0:QB + q_len], scp[:, 0:QB + q_len],
                                             mybir.ActivationFunctionType.Exp)
                        # at = ex * band  (band col c = q - kt*128), per batch
                        at = p2a.tile([128, 2 * QB], BF16, tag="at", name=f"at{qb}_{hh}_{kt}")
                        c0 = qb * QB + q_off - kt * 128
                        bb = bb_sb[hh][:, c0: c0 + q_len]
                        for b in range(B):
                            nc.vector.tensor_mul(at[:, b * QB: b * QB + q_len],
                                                 ex[:, b * QB: b * QB + q_len], bb)
                        pend = (kt, q_off, q_len, at)
                    issue_ctx(*pend)
                    # fused normalization: exact reciprocal (stalls hidden by
                    # pumped projection matmuls), broadcast, normalize+drain
                    for b in range(B):
                        cp = ctx_ps[b]
                        rcp = p2r.tile([1, QB], F32, tag=f"rc{b}", name=f"rc{qb}{hh}{b}")
                        nc.vector.reciprocal(rcp, cp[DK:DK + 1, :])
                        rbs = p2r.tile([DK, QB], F32, tag=f"rb{b}", name=f"rbs{qb}{hh}{b}")
                        nc.gpsimd.partition_broadcast(rbs, rcp)
                        nc.vector.tensor_mul(csc[b][hh * DK:(hh + 1) * DK, :],
                                             cp[0:DK, :], rbs)
                # flush any remaining units
                while popped[0] < len(units):
                    units[popped[0]]()
                    popped[0] += 1
                return csc

            # ---------------- output projection of attention block qb ----------------
            def emit_qb_out(qb, csc):
                for b in range(B):
                    for rs in range(QB // 128):
                        ot = p2o.tile([128, D], BF16, tag="ot")
                        for eh in range(2):
                            op = psh.tile([128, 512], F32, tag="sh", name=f"op{qb}{b}{rs}{eh}")
                            nc.tensor.matmul(op, csc[b][:, rs * 128:(rs + 1) * 128],
                                             wo_sb[:, eh * 512:(eh + 1) * 512],
                                             start=True, stop=True)
                            if eh == 0:
                                nc.vector.tensor_copy(ot[:, 0:512], op)
                            else:
                                nc.scalar.copy(ot[:, 512:1024], op)
                        nc.gpsimd.dma_start(
                            out=out[b * S + qb * QB + rs * 128: b * S + qb * QB + (rs + 1) * 128, :],
                            in_=ot)

            # ---------------- interleaved emission with fine-grained pumping ----------------
            for u in make_group_units(0, 0) + make_group_units(1, 0):
                u()
            for j in range(NSEG):
                if j + 1 < NSEG:
                    units = make_group_units(0, j + 1) + make_group_units(1, j + 1)
                else:
                    units = []
                csc = emit_qb_attn(j, units)
                emit_qb_out(j, csc)

    nc.compile()
    return nc


_CACHE = {}


def _get_kernel(causal: bool, qk_bias: bool):
    key = (causal, qk_bias)
    if key not in _CACHE:
        _CACHE[key] = _build(causal, qk_bias)
    return _CACHE[key]


# RoPE pair deinterleave: within each head's 64 features, evens first then odds
_PERM64 = np.concatenate([np.arange(0, 64, 2), np.arange(1, 64, 2)])


def _head_pair(c):
    """Heads assigned to core c: (full-window head, fast-decay head)."""
    return (8 + c, c)


def _host_prep(x, mask, bias, rope_freqs, Wq, bq, Wk, bk, Wv, bv, Wo, bo, causal):
    bf = ml_dtypes.bfloat16
    xf = np.ascontiguousarray(x.reshape(R, D).T.astype(bf))  # (D, R)
    cosf = np.cos(rope_freqs.astype(np.float32))  # (S, 32)
    sinf = np.sin(rope_freqs.astype(np.float32))
    rr = np.arange(R)
    cs_full = cosf[rr % S]  # (R, 32)
    sn_full = sinf[rr % S]
    cosp = np.ascontiguousarray(
        cs_full.reshape(RT, 128, 32).transpose(1, 0, 2).reshape(128, RT * 32).astype(bf))
    sinp = np.ascontiguousarray(
        sn_full.reshape(RT, 128, 32).transpose(1, 0, 2).reshape(128, RT * 32).astype(bf))
    identm = np.eye(128, dtype=np.float32).astype(bf)

    qk_bias = bool(np.any(bq) or np.any(bk))
    kl = np.arange(128, dtype=np.float64)[:, None]

    in_maps = []
    for c in range(NCORES):
        heads = _head_pair(c)
        rows = np.concatenate([h * DK + np.arange(DK) for h in heads])
        rows_perm = np.concatenate([h * DK + _PERM64 for h in heads])
        wq = Wq[rows_perm, :].astype(np.float32) / np.sqrt(np.float32(DK))
        wk = Wk[rows_perm, :].astype(np.float32)
        wv = Wv[rows, :].astype(np.float32)
        wcat = np.ascontiguousarray(np.concatenate([wq, wk, wv], axis=0).T.astype(bf))
        wob = np.ascontiguousarray(Wo[:, rows].T.astype(bf))  # (128, D)
        bbt = np.empty((HPC, 128, S), dtype=bf)
        for jh, h in enumerate(heads):
            sl_ = float(bias[h, 0, 1] - bias[h, 0, 0])  # ALiBi slope of head h
            cq = np.arange(S, dtype=np.float64)[None, :]
            val = np.exp(np.minimum(sl_ * (kl - cq), 80.0))
            val[kl > cq] = 0.0
            bbt[jh] = val.astype(bf)
        m = {
            "xT": xf, "wcat": wcat, "woB": wob,
            "bband": bbt, "cosp": cosp, "sinp": sinp, "ident": identm,
            "identf": np.eye(128, dtype=np.float32),
        }
        if qk_bias:
            for name, bvec in (("bropeq", bq / np.sqrt(np.float32(DK))), ("bropek", bk)):
                bt = np.empty((128, R), dtype=np.float32)
                for jh, h in enumerate(heads):
                    hv = bvec[h * DK:(h + 1) * DK].astype(np.float32).reshape(DK // 2, 2)
                    be = hv[:, 0][None, :]
                    bo_ = hv[:, 1][None, :]
                    rot_e = be * cs_full - bo_ * sn_full
                    rot_o = be * sn_full + bo_ * cs_full
                    blk = np.empty((R, DK), dtype=np.float32)
                    blk[:, 0:32] = rot_e
                    blk[:, 32:64] = rot_o
                    bt[jh * DK:(jh + 1) * DK, :] = blk.T
                m[name] = np.ascontiguousarray(bt.astype(bf))
        in_maps.append(m)
    return in_maps, qk_bias


def kernel(x, mask, bias, rope_freqs, Wq, bq, Wk, bk, Wv, bv, Wo, bo, **extra):
    x = np.asarray(x); mask = np.asarray(mask); bias = np.asarray(bias)
    rope_freqs = np.asarray(rope_freqs)
    Wq = np.asarray(Wq); bq = np.asarray(bq); Wk = np.asarray(Wk); bk = np.asarray(bk)
    Wv = np.asarray(Wv); bv = np.asarray(bv); Wo = np.asarray(Wo); bo = np.asarray(bo)

    causal = bool(np.array_equal(mask != 0, np.tril(np.ones((S, S), dtype=bool))))
    in_maps, qk_bias = _host_prep(x, mask, bias, rope_freqs, Wq, bq, Wk, bk, Wv, bv,
                                  Wo, bo, causal)
    nc = _get_kernel(causal, qk_bias)
    res = run_bass_kernel_spmd(nc, in_maps, list(range(NCORES)))
    acc = np.zeros((R, D), dtype=np.float32)
    for c in range(NCORES):
        acc += res.results[c]["out"].astype(np.float32)
    acc += bo.astype(np.float32)[None, :]
    if np.any(bv):
        acc += (bv.astype(np.float32) @ Wo.T.astype(np.float32))[None, :]
    return acc.reshape(B, S, D).astype(np.float32)
